# revision 11
# baseline (speedup 1.0000x reference)
"""Trainium2 Bass kernel for a 6-layer transformer decoder (self+cross attention).

Sharding: pure data parallelism - 32 batch elements / 8 cores = 4 per core.
Each core runs the full decoder on its shard; no collectives.

Device layout: activations are "feature-major" ([feature, token], feature on
SBUF partitions) so every projection matmul consumes weights in their natural
[in_feature, out_feature] layout.  LayerNorm gain/bias are folded into the
following projection weights on the host; the per-token standardization
(x - mu) * rstd is applied with PE-broadcast rows + DVE (or folded into the
projection as a rank-1 correction).  Softmax row sums are folded into the PE
transpose of the attention probabilities by using diag(1/rowsum) as the
transpose "identity".

Dtypes: fp32r (full-rate, TF32-like) for projection/FF matmuls; bf16 for
attention internals (q/k/v representations, probabilities) and the second FF
matmul.  PSUM accumulation is always fp32.
"""

import math

import numpy as np
import ml_dtypes

import concourse.bacc as bacc
import concourse.mybir as mybir
import concourse.tile as tile
from concourse.bass_utils import run_bass_kernel_spmd

# ---- problem dims (hardcoded per spec) ----
V, D, A, H, FF, L, MD = 500, 512, 512, 8, 2048, 6, 512
HD = A // H                      # 64
B, S, T = 32, 512, 128
NCORES = 8
BPC = B // NCORES                # 4 batch elements per core
NT = BPC * T                     # 512 target tokens per core
NS = BPC * S                     # 2048 memory tokens per core
VP = 512                         # padded vocab
NEG = -30000.0                   # additive mask value
EPS = 1e-5

F32 = mybir.dt.float32
F32R = mybir.dt.float32r
BF16 = mybir.dt.bfloat16
AX = mybir.AluOpType
AF = mybir.ActivationFunctionType

BF16NP = ml_dtypes.bfloat16

# column-bias layout inside the packed [128, 40] bcol tile
BQ, BK, BO, BKC, BOC, B2, B1 = 0, 4, 8, 12, 16, 20, 24


def _build_program(num_layers=L):
    nc = bacc.Bacc("TRN2", target_bir_lowering=False, debug=False,
                   num_devices=NCORES)

    def din(name, shape, dtype=F32R):
        return nc.dram_tensor(name, shape, dtype, kind="ExternalInput").ap()

    dram = dict(
        x0T=din("x0T", [D, NT]),
        memT=din("memT", [MD, NS]),
        sab=din("sa_bias", [T, NT], BF16),          # [t, b*T']
        cab=din("ca_bias", [T, NS], BF16),          # [t, b*S]
        ident=din("ident", [128, 128], BF16),
        ones=din("ones", [128, NT]),
        wsa=din("wsa", [num_layers, D, 4 * A]),     # [wq'|wk'|wv'|wo]
        wca=din("wca", [num_layers, D, 4 * A]),     # [wqc'|wkc|wvc|woc]
        wf1=din("wf1", [num_layers, D, FF]),
        wf2=din("wf2", [num_layers, 4, 128, 4 * D], BF16),
        bcol=din("bcol", [num_layers, 128, 40], F32),
        brow=din("brow", [num_layers, 1, 2048]),
        bout=din("bout_row", [1, VP]),
        wout=din("wout", [D, VP]),
        out=nc.dram_tensor("out", [NT, VP], F32, kind="ExternalOutput").ap(),
    )

    with nc.allow_low_precision(reason="fp32r/bf16 rounding is intentional"):
        with tile.TileContext(nc) as tc:
            _emit(nc, tc, dram, num_layers)
    nc.compile()
    return nc


def _emit(nc, tc, dram, num_layers):
    cp = tc.alloc_tile_pool(name="const", bufs=1)
    wp = tc.alloc_tile_pool(name="wts", bufs=2)
    ip = tc.alloc_tile_pool(name="inter", bufs=1)   # per-layer intermediates
    ap_ = tc.alloc_tile_pool(name="attn", bufs=3)   # attention small tiles
    sp = tc.alloc_tile_pool(name="small", bufs=1)   # stats etc
    psg = tc.alloc_tile_pool(name="psg", bufs=2, space="PSUM")

    # ---- constants / state ----
    ident = cp.tile([128, 128], BF16, name="ident")
    nc.sync.dma_start(ident, dram["ident"])
    sab = cp.tile([T, NT], BF16, name="sab")
    nc.sync.dma_start(sab, dram["sab"])
    cab = cp.tile([T, NS], BF16, name="cab")
    nc.sync.dma_start(cab, dram["cab"])
    ones_sb = cp.tile([128, NT], F32R, name="ones_sb")
    nc.sync.dma_start(ones_sb, dram["ones"])
    ones_col = ones_sb[:, 0:1]
    ones_p0 = ones_sb[0:1, 0:128]
    ones_row = ones_sb[0:1, :]

    memT = [cp.tile([128, NS], F32R, name=f"memT{i}") for i in range(4)]
    for i in range(4):
        nc.sync.dma_start(memT[i], dram["memT"][i * 128:(i + 1) * 128, :])
    x = [cp.tile([128, NT], F32R, name=f"x{i}") for i in range(4)]
    for i in range(4):
        nc.sync.dma_start(x[i], dram["x0T"][i * 128:(i + 1) * 128, :])

    # ---------------- helpers ----------------
    def ln_stats(tag):
        """Per-token alpha=rstd and beta=-mu*rstd rows [1, NT] (f32r)."""
        sums = []
        for which in range(2):                      # 0: sum x, 1: sum x^2
            ps = psg.tile([1, NT], F32, name=f"st{which}_{tag}", tag="proj")
            if which == 1:
                for i in range(4):
                    sq = ip.tile([128, NT], F32R, name=f"sq{i}_{tag}",
                                 tag=f"xh{i}")
                    nc.vector.tensor_mul(sq, x[i].bitcast(F32),
                                         x[i].bitcast(F32))
                    nc.tensor.matmul(ps, ones_col, sq,
                                     start=(i == 0), stop=(i == 3))
            else:
                for i in range(4):
                    nc.tensor.matmul(ps, ones_col, x[i],
                                     start=(i == 0), stop=(i == 3))
            sb = sp.tile([1, NT], F32, name=f"stsb{which}_{tag}",
                         tag=f"stsb{which}")
            nc.vector.tensor_scalar_mul(sb, ps, 1.0 / D)
            sums.append(sb)
        mean, msq = sums
        # alpha/beta tiles double as scratch: alpha <- mean^2, var, rstd;
        # beta <- std, then -mean*rstd.
        alpha = sp.tile([1, NT], F32R, name=f"alpha_{tag}", tag="alpha")
        beta = sp.tile([1, NT], F32R, name=f"beta_{tag}", tag="beta")
        var = sp.tile([1, NT], F32, name=f"var_{tag}", tag="var")
        nc.vector.tensor_mul(var, mean, mean)
        # var = (msq + eps) - mean^2
        nc.vector.scalar_tensor_tensor(var, msq, EPS, var,
                                       op0=AX.add, op1=AX.subtract)
        nc.scalar.activation(msq, var, AF.Sqrt)     # msq is dead: reuse as std
        nc.vector.reciprocal(alpha, msq)
        nc.vector.scalar_tensor_tensor(beta, mean, -1.0, alpha.bitcast(F32),
                                       op0=AX.mult, op1=AX.mult)
        return alpha, beta

    def broadcast_row(row, tag):
        """PE-broadcast a [1, NT] f32r row to a [128, NT] f32 sbuf tile."""
        ps = psg.tile([128, NT], F32, name=f"bc_ps_{tag}", tag="proj")
        nc.tensor.matmul(ps, ones_p0, row, start=True, stop=True)
        out = sp.tile([128, NT], F32, name=f"bcast_{tag}", tag=f"bc_{tag}")
        nc.vector.tensor_copy(out, ps)
        return out

    def materialize_xhat(alpha_b, beta_b, tag):
        xh = [ip.tile([128, NT], F32R, name=f"xh{i}_{tag}", tag=f"xh{i}")
              for i in range(4)]
        for i in range(4):
            nc.vector.tensor_mul(xh[i], x[i].bitcast(F32), alpha_b)
            nc.vector.tensor_add(xh[i], xh[i].bitcast(F32), beta_b)
        return xh

    def load_w(dram_w, l, tag):
        tiles = [wp.tile([128, 2048], F32R, name=f"{tag}{i}", tag=f"wB{i}")
                 for i in range(4)]
        for i in range(4):
            nc.sync.dma_start(tiles[i], dram_w[l, i * 128:(i + 1) * 128, :])
        return tiles

    def softmax_T(s_ps, bias_ap, width, nm):
        """bias-add + exp + rowsum normalization + PE transpose.
        Returns at: [128, width] bf16 sbuf tile holding A^T (t' major)."""
        tmp = ap_.tile([128, width], F32, name=f"tmp{nm}", tag="tmp", bufs=2)
        nc.vector.tensor_add(tmp, s_ps, bias_ap)
        ex = ap_.tile([128, width], F32, name=f"ex{nm}", tag="ex")
        rsum = ap_.tile([128, 1], F32, name=f"rs{nm}", tag="rsum")
        nc.scalar.activation(ex, tmp, AF.Exp, accum_out=rsum)
        rcp = ap_.tile([128, 1], F32, name=f"rc{nm}", tag="rcp")
        nc.vector.reciprocal(rcp, rsum)
        exn = ap_.tile([128, width], BF16, name=f"exn{nm}", tag="exn")
        nc.vector.tensor_scalar_mul(exn, ex, rcp)
        at = ap_.tile([128, width], BF16, name=f"at{nm}", tag="at")
        for sc in range(width // 128):
            at_ps = att_pool.tile([128, T], BF16, name=f"atp{nm}{sc}",
                                  tag="att")
            nc.tensor.transpose(at_ps, exn[:, sc * 128:sc * 128 + 128],
                                ident)
            nc.vector.tensor_copy(at[:, sc * T:(sc + 1) * T], at_ps)
        return at

    # ---------------- layers ----------------
    for l in range(num_layers):
        # ======== self-attention ========
        wsa = load_w(dram["wsa"], l, "sa")
        bcol = wp.tile([128, 40], F32, name="bcol", tag="bcol")
        nc.sync.dma_start(bcol, dram["bcol"][l])
        brow = wp.tile([1, 2048], F32R, name="brow", tag="brow", bufs=1)
        nc.sync.dma_start(brow, dram["brow"][l])

        alpha, beta = ln_stats(f"sa{l}")
        alpha_b = broadcast_row(alpha, "a")
        beta_b = broadcast_row(beta, "b")
        xh = materialize_xhat(alpha_b, beta_b, f"sa{l}")

        # q', k' feature-major (bf16), v token-major (bf16)
        qp, kp = [], []
        for kind, off, bco, dst in (("qp", 0, BQ, qp), ("kp", A, BK, kp)):
            for ac in range(4):
                ps = psg.tile([128, NT], F32, name=f"{kind}_ps{ac}",
                              tag="proj")
                for dc in range(4):
                    nc.tensor.matmul(
                        ps, wsa[dc][:, off + ac * 128:off + ac * 128 + 128],
                        xh[dc], start=(dc == 0), stop=(dc == 3))
                t = ip.tile([128, NT], BF16, name=f"{kind}{ac}_{l}",
                            tag=f"{kind}{ac}")
                nc.vector.tensor_scalar_add(t, ps, bcol[:, bco + ac:
                                                        bco + ac + 1])
                dst.append(t)

        vtm = []
        for b in range(BPC):
            ps = psg.tile([128, A], F32, name=f"v_ps{b}", tag="proj")
            for dc in range(4):
                nc.tensor.matmul(ps, xh[dc][:, b * 128:(b + 1) * 128],
                                 wsa[dc][:, 2 * A:3 * A],
                                 start=(dc == 0), stop=False)
            nc.tensor.matmul(ps, ones_p0, brow[0:1, 1024:1536],
                             start=False, stop=True)
            vt = ip.tile([128, A], BF16, name=f"vtm{b}", tag=f"vtm{b}")
            nc.vector.tensor_copy(vt, ps)
            vtm.append(vt)

        # attention per (b, head-pair)
        att_pool = tc.alloc_tile_pool(name="sa_att", bufs=4, space="PSUM")
        osb = [ip.tile([128, NT], F32R, name=f"osb{hp}_{l}", tag=f"osb{hp}")
               for hp in range(4)]
        for b in range(BPC):
            for hp in range(4):
                at_sb = []
                for e in range(2):
                    lo = e * 64
                    s_ps = att_pool.tile([128, T], F32, name=f"s{b}{hp}{e}",
                                         tag="att")
                    nc.tensor.matmul(
                        s_ps,
                        qp[hp][lo:lo + 64, b * T:(b + 1) * T],
                        kp[hp][lo:lo + 64, b * T:(b + 1) * T],
                        start=True, stop=True)
                    at_sb.append(softmax_T(s_ps, sab[:, b * T:(b + 1) * T],
                                           T, f"s{b}{hp}{e}"))
                o_ps = att_pool.tile([128, T], F32, name=f"o{b}{hp}",
                                     tag="att")
                for e in range(2):
                    h = 2 * hp + e
                    nc.tensor.matmul(o_ps[e * 64:(e + 1) * 64, :],
                                     vtm[b][:, h * HD:(h + 1) * HD],
                                     at_sb[e], start=True, stop=True)
                nc.vector.tensor_copy(osb[hp][:, b * T:(b + 1) * T], o_ps)
        att_pool.release()

        # out projection + residual
        for dc in range(4):
            ps = psg.tile([128, NT], F32, name=f"op{dc}", tag="proj")
            for ac in range(4):
                nc.tensor.matmul(ps, wsa[ac][:, 3 * A + dc * 128:
                                             3 * A + dc * 128 + 128],
                                 osb[ac], start=(ac == 0), stop=(ac == 3))
            nc.vector.scalar_tensor_tensor(
                x[dc], ps, bcol[:, BO + dc:BO + dc + 1], x[dc].bitcast(F32),
                op0=AX.add, op1=AX.add)

        # ======== cross-attention ========
        wca = load_w(dram["wca"], l, "ca")
        alpha2, beta2 = ln_stats(f"ca{l}")
        alpha2_b = broadcast_row(alpha2, "a")

        qc = []
        for ac in range(4):
            ps = psg.tile([128, NT], F32, name=f"qc_ps{ac}", tag="proj")
            for dc in range(4):
                nc.tensor.matmul(ps, wca[dc][:, ac * 128:ac * 128 + 128],
                                 x[dc], start=(dc == 0), stop=False)
            # LN fold corrections: beta2[t]*colsum(wqc)[a] + bqc[a]*1
            nc.tensor.matmul(ps, brow[0:1, ac * 128:ac * 128 + 128], beta2,
                             start=False, stop=False)
            nc.tensor.matmul(ps, brow[0:1, 512 + ac * 128:512 + ac * 128 + 128],
                             ones_row, start=False, stop=True)
            q = ip.tile([128, NT], BF16, name=f"qc{ac}", tag=f"qp{ac}")
            nc.vector.tensor_mul(q, ps, alpha2_b)
            qc.append(q)

        att_pool = tc.alloc_tile_pool(name="ca_att", bufs=4, space="PSUM")
        ocb = [ip.tile([128, NT], F32R, name=f"ocb{hp}_{l}", tag=f"osb{hp}")
               for hp in range(4)]
        for b in range(BPC):
            kc = []
            for ac in range(4):
                ps = psg.tile([128, S], F32, name=f"kc_ps{ac}", tag="proj")
                for dc in range(4):
                    nc.tensor.matmul(
                        ps, wca[dc][:, A + ac * 128:A + ac * 128 + 128],
                        memT[dc][:, b * S:(b + 1) * S],
                        start=(dc == 0), stop=(dc == 3))
                k = ip.tile([128, S], BF16, name=f"kc{ac}", tag=f"kc{ac}")
                nc.vector.tensor_scalar_add(k, ps,
                                            bcol[:, BKC + ac:BKC + ac + 1])
                kc.append(k)
            vc = []
            for sc in range(4):
                ps = psg.tile([128, A], F32, name=f"vc_ps{sc}", tag="proj")
                for dc in range(4):
                    nc.tensor.matmul(
                        ps,
                        memT[dc][:, b * S + sc * 128:b * S + sc * 128 + 128],
                        wca[dc][:, 2 * A:3 * A],
                        start=(dc == 0), stop=False)
                nc.tensor.matmul(ps, ones_p0, brow[0:1, 1536:2048],
                                 start=False, stop=True)
                v = ip.tile([128, A], BF16, name=f"vc{sc}", tag=f"vc{sc}")
                nc.vector.tensor_copy(v, ps)
                vc.append(v)

            for hp in range(4):
                at_sb = []
                for e in range(2):
                    lo = e * 64
                    s_ps = psg.tile([128, S], F32, name=f"cs{b}{hp}{e}",
                                    tag="proj")
                    nc.tensor.matmul(s_ps,
                                     qc[hp][lo:lo + 64, b * T:(b + 1) * T],
                                     kc[hp][lo:lo + 64, :],
                                     start=True, stop=True)
                    at_sb.append(softmax_T(s_ps, cab[:, b * S:(b + 1) * S],
                                           S, f"c{b}{hp}{e}"))
                o_ps = att_pool.tile([128, T], F32, name=f"co{b}{hp}",
                                     tag="att")
                for e in range(2):
                    h = 2 * hp + e
                    for sc in range(4):
                        nc.tensor.matmul(
                            o_ps[e * 64:(e + 1) * 64, :],
                            vc[sc][:, h * HD:(h + 1) * HD],
                            at_sb[e][:, sc * T:(sc + 1) * T],
                            start=(sc == 0), stop=(sc == 3))
                nc.vector.tensor_copy(ocb[hp][:, b * T:(b + 1) * T], o_ps)
        att_pool.release()

        for dc in range(4):
            ps = psg.tile([128, NT], F32, name=f"ocp{dc}", tag="proj")
            for ac in range(4):
                nc.tensor.matmul(ps, wca[ac][:, 3 * A + dc * 128:
                                             3 * A + dc * 128 + 128],
                                 ocb[ac], start=(ac == 0), stop=(ac == 3))
            nc.vector.scalar_tensor_tensor(
                x[dc], ps, bcol[:, BOC + dc:BOC + dc + 1], x[dc].bitcast(F32),
                op0=AX.add, op1=AX.add)

        # ======== feed-forward ========
        wf1 = load_w(dram["wf1"], l, "f1")
        alpha3, beta3 = ln_stats(f"ff{l}")
        alpha3_b = broadcast_row(alpha3, "a")
        beta3_b = broadcast_row(beta3, "b")
        xh3 = materialize_xhat(alpha3_b, beta3_b, f"ff{l}")

        ff_pool = tc.alloc_tile_pool(name="ff_acc", bufs=4, space="PSUM")
        facc = [ff_pool.tile([128, NT], F32, name=f"facc{dc}", tag="ffacc")
                for dc in range(4)]
        for j in range(4):
            w2t = wp.tile([128, 4 * D], BF16, name=f"w2t{j}", tag="w2t",
                          bufs=2)
            nc.sync.dma_start(w2t, dram["wf2"][l, j])
            for i in range(4):
                fc = 4 * j + i
                ps = psg.tile([128, NT], F32, name=f"h_ps{fc}", tag="proj")
                for dc in range(4):
                    nc.tensor.matmul(ps, wf1[dc][:, fc * 128:fc * 128 + 128],
                                     xh3[dc], start=(dc == 0), stop=(dc == 3))
                # swish(ps + b1) = (ps + b1) * sigmoid(ps + b1)
                sg = ap_.tile([128, NT], F32, name=f"sg{fc}", tag="tmp",
                              bufs=2)
                nc.scalar.activation(sg, ps, AF.Sigmoid,
                                     bias=bcol[:, B1 + fc:B1 + fc + 1])
                h = ip.tile([128, NT], BF16, name=f"h{fc}", tag="hstream",
                            bufs=2)
                nc.vector.scalar_tensor_tensor(
                    h, ps, bcol[:, B1 + fc:B1 + fc + 1], sg,
                    op0=AX.add, op1=AX.mult)
                for dc in range(4):
                    nc.tensor.matmul(
                        facc[dc],
                        w2t[:, i * D + dc * 128:i * D + dc * 128 + 128],
                        h, start=(fc == 0), stop=(fc == 15))
        for dc in range(4):
            nc.vector.scalar_tensor_tensor(
                x[dc], facc[dc], bcol[:, B2 + dc:B2 + dc + 1],
                x[dc].bitcast(F32), op0=AX.add, op1=AX.add)
        ff_pool.release()

    # ======== final projection ========
    wout = wp.tile([128, 2048], F32R, name="wout", tag="wB0")
    for i in range(4):
        nc.sync.dma_start(wout[:, i * 512:(i + 1) * 512],
                          dram["wout"][i * 128:(i + 1) * 128, :])
    boutr = wp.tile([1, VP], F32R, name="boutr", tag="boutr")
    nc.sync.dma_start(boutr, dram["bout"])
    for tcb in range(4):
        ps = psg.tile([128, VP], F32, name=f"lg{tcb}", tag="proj")
        for dc in range(4):
            nc.tensor.matmul(ps, x[dc][:, tcb * 128:(tcb + 1) * 128],
                             wout[:, dc * 512:(dc + 1) * 512],
                             start=(dc == 0), stop=False)
        nc.tensor.matmul(ps, ones_p0, boutr, start=False, stop=True)
        lg = sp.tile([128, VP], F32, name=f"lgs{tcb}", tag="lgs")
        nc.vector.tensor_copy(lg, ps)
        nc.sync.dma_start(dram["out"][tcb * 128:(tcb + 1) * 128, :], lg)

    for p in (psg, sp, ap_, ip, wp, cp):
        p.release()


# ---------------- host side ----------------

def _host_prep(inputs, num_layers=L):
    f32 = np.float32
    memory = np.asarray(inputs["memory"], f32)                 # (B, S, MD)
    memory_lens = np.asarray(inputs["memory_lens"]).astype(np.int64)
    ys = np.asarray(inputs["ys_in_pad"]).astype(np.int64)      # (B, T)
    ys_lens = np.asarray(inputs["ys_in_lens"]).astype(np.int64)
    emb = np.asarray(inputs["emb"], f32)
    w_out = np.asarray(inputs["w_out"], f32)
    b_out = np.asarray(inputs["b_out"], f32)

    g = {k: np.asarray(inputs[k], f32) for k in (
        "ln_sa_g", "ln_sa_b", "sa_wq", "sa_bq", "sa_wk", "sa_bk", "sa_wv",
        "sa_bv", "sa_wo", "sa_bo", "ln_ca_g", "ln_ca_b", "ca_wq", "ca_bq",
        "ca_wk", "ca_bk", "ca_wv", "ca_bv", "ca_wo", "ca_bo", "ln_ff_g",
        "ln_ff_b", "ff_w1", "ff_b1", "ff_w2", "ff_b2")}

    # positional encoding + embedding
    pos = np.arange(T, dtype=f32)[:, None]
    div = np.exp(np.arange(0, D, 2, dtype=f32) * (-math.log(10000.0) / D))
    pe = np.zeros((T, D), f32)
    pe[:, 0::2] = np.sin(pos * div)
    pe[:, 1::2] = np.cos(pos * div)
    x0 = emb[ys] * math.sqrt(D) + pe[None]                     # (B, T, D)

    # masks as additive biases
    t_idx = np.arange(T)
    s_idx = np.arange(S)
    pad = t_idx[None, :] >= ys_lens[:, None]                   # (B, T') keys
    causal = t_idx[None, :] > t_idx[:, None]                   # (T, T')
    tgt_mask = pad[:, None, :] | causal[None]                  # (B, T, T')
    sa_bias = np.where(tgt_mask, NEG, 0.0).astype(f32)
    mem_mask = s_idx[None, :] >= memory_lens[:, None]          # (B, S)
    ca_bias = np.where(mem_mask, NEG, 0.0).astype(f32)
    ca_bias = np.broadcast_to(ca_bias[:, None, :], (B, T, S))

    # per-layer folded weights
    qscale = 1.0 / math.sqrt(HD)
    wsa = np.empty((num_layers, D, 4 * A), f32)
    wca = np.empty((num_layers, D, 4 * A), f32)
    wf1 = np.empty((num_layers, D, FF), f32)
    wf2 = np.empty((num_layers, 4, 128, 4 * D), BF16NP)
    bcol = np.zeros((num_layers, 128, 40), f32)
    brow = np.zeros((num_layers, 1, 2048), f32)
    for l in range(num_layers):
        g1, b1 = g["ln_sa_g"][l], g["ln_sa_b"][l]
        wq = g1[:, None] * g["sa_wq"][l] * qscale
        bq = (b1 @ g["sa_wq"][l] + g["sa_bq"][l]) * qscale
        wk = g1[:, None] * g["sa_wk"][l]
        bk = b1 @ g["sa_wk"][l] + g["sa_bk"][l]
        wv = g1[:, None] * g["sa_wv"][l]
        bv = b1 @ g["sa_wv"][l] + g["sa_bv"][l]
        wsa[l] = np.concatenate([wq, wk, wv, g["sa_wo"][l]], axis=1)
        g2, b2 = g["ln_ca_g"][l], g["ln_ca_b"][l]
        wqc = g2[:, None] * g["ca_wq"][l] * qscale
        bqc = (b2 @ g["ca_wq"][l] + g["ca_bq"][l]) * qscale
        wca[l] = np.concatenate([wqc, g["ca_wk"][l], g["ca_wv"][l],
                                 g["ca_wo"][l]], axis=1)
        g3, b3 = g["ln_ff_g"][l], g["ln_ff_b"][l]
        wf1[l] = g3[:, None] * g["ff_w1"][l]
        b1f = b3 @ g["ff_w1"][l] + g["ff_b1"][l]
        wf2[l] = g["ff_w2"][l].reshape(4, 4, 128, D).transpose(
            0, 2, 1, 3).reshape(4, 128, 4 * D).astype(BF16NP)

        for ac in range(4):
            sl = slice(ac * 128, (ac + 1) * 128)
            bcol[l, :, BQ + ac] = bq[sl]
            bcol[l, :, BK + ac] = bk[sl]
            bcol[l, :, BO + ac] = g["sa_bo"][l][sl]
            bcol[l, :, BKC + ac] = g["ca_bk"][l][sl]
            bcol[l, :, BOC + ac] = g["ca_bo"][l][sl]
            bcol[l, :, B2 + ac] = g["ff_b2"][l][sl]
        for fc in range(16):
            bcol[l, :, B1 + fc] = b1f[fc * 128:(fc + 1) * 128]
        brow[l, 0, 0:512] = wqc.sum(axis=0)
        brow[l, 0, 512:1024] = bqc
        brow[l, 0, 1024:1536] = bv
        brow[l, 0, 1536:2048] = g["ca_bv"][l]

    wout_pad = np.zeros((D, VP), f32)
    wout_pad[:, :V] = w_out
    bout_pad = np.zeros((1, VP), f32)
    bout_pad[0, :V] = b_out

    shared = dict(ident=np.eye(128, dtype=BF16NP),
                  ones=np.ones((128, NT), np.float32),
                  wsa=wsa, wca=wca, wf1=wf1, wf2=wf2,
                  bcol=bcol, brow=brow, wout=wout_pad, bout_row=bout_pad)

    in_maps = []
    for c in range(NCORES):
        bs = slice(c * BPC, (c + 1) * BPC)
        m = dict(shared)
        m["x0T"] = np.ascontiguousarray(
            x0[bs].transpose(2, 0, 1).reshape(D, NT))
        m["memT"] = np.ascontiguousarray(
            memory[bs].transpose(2, 0, 1).reshape(MD, NS))
        m["sa_bias"] = np.ascontiguousarray(
            sa_bias[bs].transpose(1, 0, 2).reshape(T, NT)).astype(BF16NP)
        m["ca_bias"] = np.ascontiguousarray(
            ca_bias[bs].transpose(1, 0, 2).reshape(T, NS)).astype(BF16NP)
        in_maps.append(m)
    return in_maps


_PROGRAM = None


def _get_program():
    global _PROGRAM
    if _PROGRAM is None:
        _PROGRAM = _build_program()
    return _PROGRAM


def kernel(**inputs):
    nc = _get_program()
    in_maps = _host_prep(inputs)
    res = run_bass_kernel_spmd(nc, in_maps, core_ids=list(range(NCORES)))
    outs = []
    for c in range(NCORES):
        o = res.results[c]["out"]                # (NT, VP)
        outs.append(o.reshape(BPC, T, VP)[:, :, :V])
    return np.ascontiguousarray(
        np.concatenate(outs, axis=0).astype(np.float32))


if __name__ == "__main__":
    nc = _build_program()
    print("built ok; instructions:",
          sum(len(b.instructions) for b in nc.main_func.blocks))
